# revision 1
# baseline (speedup 1.0000x reference)
import numpy as np

import concourse.bacc as bacc
import concourse.bass as bass
import concourse.mybir as mybir
import concourse.tile as tile
from concourse.bass_utils import run_bass_kernel_spmd
from concourse.masks import make_identity

B, Q, KL, D = 32, 8, 2048, 256
H, DH = 8, 32
NCORES = 8
NB = B // NCORES
KC = KL // 128
F32 = mybir.dt.float32
BF16 = mybir.dt.bfloat16
Tanh = mybir.ActivationFunctionType.Tanh
Exp = mybir.ActivationFunctionType.Exp


def _emit(tc):
    nc = tc.nc

    queries = nc.dram_tensor("queries", [NB, Q, D], F32, kind="ExternalInput").ap()
    keys = nc.dram_tensor("keys", [NB, KL, D], F32, kind="ExternalInput").ap()
    values = nc.dram_tensor("values", [NB, KL, D], F32, kind="ExternalInput").ap()
    Wq = nc.dram_tensor("Wq", [D, D], F32, kind="ExternalInput").ap()
    Wk = nc.dram_tensor("Wk", [D, D], F32, kind="ExternalInput").ap()
    Wv = nc.dram_tensor("Wv", [D, D], F32, kind="ExternalInput").ap()
    Wo = nc.dram_tensor("Wo", [D, D], F32, kind="ExternalInput").ap()
    wv_score = nc.dram_tensor("wv_score", [DH], F32, kind="ExternalInput").ap()
    fcW = nc.dram_tensor("fcW", [D, Q * D], F32, kind="ExternalInput").ap()
    fcb = nc.dram_tensor("fcb", [D], F32, kind="ExternalInput").ap()
    out = nc.dram_tensor("out", [NB, D], F32, kind="ExternalOutput").ap()

    dram = tc.alloc_tile_pool(name="dram", bufs=1, space="DRAM")
    consts = tc.alloc_tile_pool(name="consts", bufs=1)
    psA = tc.alloc_tile_pool(name="psA", bufs=1, space="PSUM")
    vp_pool = tc.alloc_tile_pool(name="vp_ps", bufs=2, space="PSUM")
    krep_pool = tc.alloc_tile_pool(name="krep_ps", bufs=1, space="PSUM")
    sc_pool = tc.alloc_tile_pool(name="sc_ps", bufs=2, space="PSUM")
    ao_pool = tc.alloc_tile_pool(name="ao_ps", bufs=1, space="PSUM")
    krepsb_pool = tc.alloc_tile_pool(name="krep_sb", bufs=4)
    feat_pool = tc.alloc_tile_pool(name="feat", bufs=4)
    soft_pool = tc.alloc_tile_pool(name="soft", bufs=2)
    pools = [
        soft_pool, feat_pool, krepsb_pool, ao_pool, sc_pool,
        krep_pool, vp_pool, psA, consts, dram,
    ]

    id32b = consts.tile([32, 32], BF16, tag="id32b", name="id32b")
    id32f = consts.tile([32, 32], F32, tag="id32f", name="id32f")
    make_identity(nc, id32b[:])
    make_identity(nc, id32f[:])
    dummy = consts.tile([1, 2], F32, tag="dummy", name="dummy")
    nc.vector.memset(dummy[:], 0.0)
    nc.scalar.activation(out=dummy[:], in_=dummy[:], func=Tanh)

    S_f32 = consts.tile([128, 4], F32, tag="S_f32", name="S_f32")
    S = consts.tile([128, 4], BF16, tag="S", name="S")
    nc.vector.memset(S_f32[:], 0.0)
    wv_col = wv_score.rearrange("(d one) -> d one", one=1)
    for hh in range(4):
        nc.sync.dma_start(out=S_f32[hh * 32 : (hh + 1) * 32, hh : hh + 1], in_=wv_col)
    nc.vector.tensor_copy(out=S[:], in_=S_f32[:])

    fcb_sb = consts.tile([NB, D], F32, tag="fcb_sb", name="fcb_sb")
    fcb_b = bass.AP(tensor=fcb.tensor, offset=fcb.offset, ap=[[0, NB], [1, D]])
    nc.sync.dma_start(out=fcb_sb[:], in_=fcb_b)

    wq_bf = dram.tile([D, D], BF16, tag="wq_bf", name="wq_bf")
    wk_bf = dram.tile([D, D], BF16, tag="wk_bf", name="wk_bf")
    keys_bf = dram.tile([NB, KL, D], BF16)
    nc.gpsimd.dma_start(out=wq_bf[:], in_=Wq)
    nc.gpsimd.dma_start(out=wk_bf[:], in_=Wk)
    nc.gpsimd.dma_start(out=keys_bf[0], in_=keys[0])

    def wtrans(name, src):
        ts = [consts.tile([128, D], BF16, tag=f"{name}{ch}", name=f"{name}{ch}") for ch in range(2)]
        for ch in range(2):
            nc.sync.dma_start(
                out=ts[ch][:], in_=src[:, ch * 128 : (ch + 1) * 128], transpose=True
            )
        return ts

    WqT = wtrans("WqT", wq_bf)

    q_nat = consts.tile([NB * Q, D], BF16, tag="q_nat", name="q_nat")
    nc.gpsimd.dma_start(out=q_nat[:], in_=queries.rearrange("b q d -> (b q) d"))
    qT = [consts.tile([128, NB * Q], BF16, tag=f"qT{ch}", name=f"qT{ch}") for ch in range(2)]
    for ch in range(2):
        qT_ps = psA.tile([128, NB * Q], BF16, tag="psA", name="qT_ps")
        nc.tensor.transpose(
            out=qT_ps[:], in_=q_nat[:, ch * 128 : (ch + 1) * 128], identity=id32b[:]
        )
        nc.vector.tensor_copy(out=qT[ch][:], in_=qT_ps[:])
    qpT = [consts.tile([128, NB * Q], F32, tag=f"qpT{hg}", name=f"qpT{hg}") for hg in range(2)]
    for hg in range(2):
        qpT_ps = psA.tile([128, NB * Q], F32, tag="psA", name="qpT_ps")
        for ch in range(2):
            nc.tensor.matmul(
                out=qpT_ps[:],
                lhsT=WqT[ch][:, hg * 128 : (hg + 1) * 128],
                rhs=qT[ch][:],
                start=(ch == 0),
                stop=(ch == 1),
            )
        nc.vector.tensor_copy(out=qpT[hg][:], in_=qpT_ps[:])

    wv_bf = dram.tile([D, D], BF16, tag="wv_bf", name="wv_bf")
    nc.gpsimd.dma_start(out=wv_bf[:], in_=Wv)
    WkT = wtrans("WkT", wk_bf)
    WvT = wtrans("WvT", wv_bf)

    values_bf = dram.tile([NB, KL, D], BF16)
    keysT = [
        [consts.tile([128, KL], BF16, tag=f"keysT{b}_{ch}", name=f"keysT{b}_{ch}") for ch in range(2)]
        for b in range(NB)
    ]
    valuesT = [
        [consts.tile([128, KL], BF16, tag=f"valuesT{b}_{ch}", name=f"valuesT{b}_{ch}") for ch in range(2)]
        for b in range(NB)
    ]
    v_sb = [consts.tile([128, NB * D], BF16, tag=f"v_sb{kc}", name=f"v_sb{kc}") for kc in range(KC)]
    aoT = [consts.tile([128, NB * Q], BF16, tag=f"aoT{hg}", name=f"aoT{hg}") for hg in range(2)]

    def emit_keys_chain(b, hold=None):
        if b > 0:
            cast = nc.gpsimd.dma_start(out=keys_bf[b], in_=keys[b])
            if hold is not None:
                tile.add_dep_helper(cast.ins, hold.ins, reason="dma order")
            tr = None
            for ch in range(2):
                tr = nc.sync.dma_start(
                    out=keysT[b][ch][:],
                    in_=keys_bf[b, :, ch * 128 : (ch + 1) * 128],
                    transpose=True,
                )
            return tr
        tr = None
        for ch in range(2):
            tr = nc.sync.dma_start(
                out=keysT[b][ch][:],
                in_=keys_bf[b, :, ch * 128 : (ch + 1) * 128],
                transpose=True,
            )
        return tr

    def emit_kproj(b, hg):
        krep_sb = krepsb_pool.tile([128, KL], F32, name="krep_sb")
        for half in range(2):
            krep_ps = krep_pool.tile([128, KL // 2], F32, tag="krep", name="krep_ps")
            for nch in range(2):
                nco = half * 2 + nch
                for ch in range(2):
                    nc.tensor.matmul(
                        out=krep_ps[:, nch * 512 : (nch + 1) * 512],
                        lhsT=WkT[ch][:, hg * 128 : (hg + 1) * 128],
                        rhs=keysT[b][ch][:, nco * 512 : (nco + 1) * 512],
                        start=(ch == 0),
                        stop=(ch == 1),
                    )
            nc.vector.tensor_copy(
                out=krep_sb[:, half * (KL // 2) : (half + 1) * (KL // 2)],
                in_=krep_ps[:],
            )
        return krep_sb

    def emit_values_chain(b, hold=None):
        cast = nc.gpsimd.dma_start(out=values_bf[b], in_=values[b])
        if hold is not None:
            tile.add_dep_helper(cast.ins, hold.ins, reason="dma order")
        tr = None
        for ch in range(2):
            tr = nc.sync.dma_start(
                out=valuesT[b][ch][:],
                in_=values_bf[b, :, ch * 128 : (ch + 1) * 128],
                transpose=True,
            )
        return tr

    def emit_vproj(b):
        for kc in range(KC):
            vp_ps = vp_pool.tile([128, D], F32)
            for ch in range(2):
                nc.tensor.matmul(
                    out=vp_ps[:],
                    lhsT=valuesT[b][ch][:, kc * 128 : (kc + 1) * 128],
                    rhs=WvT[ch][:],
                    start=(ch == 0),
                    stop=(ch == 1),
                )
            nc.vector.tensor_copy(out=v_sb[kc][:, b * D : (b + 1) * D], in_=vp_ps[:])

    def emit_main(b, hg, krep_sb):

        sc_ps = sc_pool.tile([128, 512], F32)
        sc_r = sc_ps[:].rearrange("p (kc q h) -> p kc q h", kc=KC, q=Q, h=4)
        for q in range(Q):
            feat = feat_pool.tile([128, KL], BF16)
            nc.scalar.activation(
                out=feat[:],
                in_=krep_sb[:],
                func=Tanh,
                bias=qpT[hg][:, b * Q + q : b * Q + q + 1],
            )
            for kc in range(KC):
                nc.tensor.matmul(
                    out=sc_r[:, kc, q, :],
                    lhsT=feat[:, kc * 128 : (kc + 1) * 128],
                    rhs=S[:],
                    start=True,
                    stop=True,
                )

        if hg == 0:
            emit_vproj(b)

        exp_sb = soft_pool.tile([128, 512], F32, tag="exp_sb", name="exp_sb")
        nc.scalar.activation(out=exp_sb[:], in_=sc_ps[:], func=Exp)
        Zt = soft_pool.tile([128, 64], F32, tag="Zt", name="Zt")
        exp_khq = exp_sb[:].rearrange("p (kc q h) -> p kc h q", kc=KC, q=Q, h=4)
        nc.vector.tensor_reduce(
            out=Zt[:], in_=exp_khq, axis=mybir.AxisListType.X, op=mybir.AluOpType.add
        )
        invZ = soft_pool.tile([128, 64], F32, tag="invZ", name="invZ")
        nc.vector.reciprocal(out=invZ[:], in_=Zt[:])
        en = soft_pool.tile([128, 512], BF16, tag="en", name="en")
        in0 = exp_sb[:].rearrange("p (kc q h) -> p kc q h", kc=KC, q=Q, h=4)
        iz = invZ[:].rearrange("p (kc h) -> p kc h", kc=KC, h=4)
        in1 = bass.AP(
            tensor=iz.tensor,
            offset=iz.offset,
            ap=[list(iz.ap[0]), list(iz.ap[1]), [0, Q], list(iz.ap[2])],
        )
        en_r = en[:].rearrange("p (kc q h) -> p kc q h", kc=KC, q=Q, h=4)
        nc.vector.tensor_tensor(out=en_r, in0=in0, in1=in1, op=mybir.AluOpType.mult)

        ao_ps = ao_pool.tile([128, Q], F32)
        prev_group_last = None
        for hh in range(4):
            j0 = b * D + (hg * 4 + hh) * DH
            for kc in range(KC):
                mm = nc.tensor.matmul(
                    out=ao_ps[hh * 32 : (hh + 1) * 32, :],
                    lhsT=v_sb[kc][:, j0 : j0 + DH],
                    rhs=en_r[:, kc, :, hh],
                    start=(kc == 0),
                    stop=(kc == KC - 1),
                    tile_position=(0, hh * 32),
                    skip_group_check=True,
                )
                if prev_group_last is not None:
                    tile.add_dep_helper(
                        mm.ins,
                        prev_group_last,
                        sync=False,
                        reason="ao accumulation group order",
                    )
                prev_group_last = mm.ins
        nc.vector.tensor_copy(out=aoT[hg][:, b * Q : (b + 1) * Q], in_=ao_ps[:])


    keys_tr = emit_keys_chain(0)
    kreps = [emit_kproj(0, 0), emit_kproj(0, 1)]
    last_tr = emit_values_chain(0, hold=keys_tr)
    for b in range(NB):
        if b + 1 < NB:
            next_keys_tr = emit_keys_chain(b + 1, hold=last_tr)
        emit_main(b, 0, kreps[0])
        if b + 1 < NB:
            next_kreps = [emit_kproj(b + 1, 0), emit_kproj(b + 1, 1)]
            last_tr = emit_values_chain(b + 1, hold=next_keys_tr)
        emit_main(b, 1, kreps[1])
        if b + 1 < NB:
            kreps = next_kreps

    wo_bf = dram.tile([D, D], BF16, tag="wo_bf", name="wo_bf")
    fcw_bf = dram.tile([D, Q * D], BF16, tag="fcw_bf", name="fcw_bf")
    wo_cast = nc.gpsimd.dma_start(out=wo_bf[:], in_=Wo)
    tile.add_dep_helper(wo_cast.ins, last_tr.ins, reason="dma order")
    fcw_cast = nc.gpsimd.dma_start(out=fcw_bf[:], in_=fcW)
    tile.add_dep_helper(fcw_cast.ins, wo_cast.ins, reason="dma order")
    WoT = wtrans("WoT", wo_bf)
    fcwT = [consts.tile([128, D], BF16, tag=f"fcwT{t}", name=f"fcwT{t}") for t in range(16)]
    for t in range(16):
        nc.sync.dma_start(
            out=fcwT[t][:], in_=fcw_bf[:, t * 128 : (t + 1) * 128], transpose=True
        )

    o2T = [consts.tile([128, NB * Q], BF16, tag=f"o2T{m}", name=f"o2T{m}") for m in range(2)]
    for m in range(2):
        o2_ps = psA.tile([128, NB * Q], F32, tag="psA", name="o2_ps")
        for ch in range(2):
            nc.tensor.matmul(
                out=o2_ps[:],
                lhsT=WoT[ch][:, m * 128 : (m + 1) * 128],
                rhs=aoT[ch][:],
                start=(ch == 0),
                stop=(ch == 1),
            )
        nc.vector.tensor_copy(out=o2T[m][:], in_=o2_ps[:])

    y_ps = psA.tile([NB, D], F32, tag="psA", name="y_ps")
    for t in range(16):
        qq, m = t // 2, t % 2
        lhsT = o2T[m][:].rearrange("p (b q) -> p q b", b=NB, q=Q)[:, qq, :]
        nc.tensor.matmul(
            out=y_ps[:], lhsT=lhsT, rhs=fcwT[t][:], start=(t == 0), stop=(t == 15)
        )
    y_sb = consts.tile([NB, D], F32, tag="y_sb", name="y_sb")
    nc.vector.tensor_tensor(
        out=y_sb[:], in0=y_ps[:], in1=fcb_sb[:], op=mybir.AluOpType.add
    )
    nc.sync.dma_start(out=out, in_=y_sb[:])

    for p in pools:
        p.release()


_NC_CACHE = None


def _get_nc():
    global _NC_CACHE
    if _NC_CACHE is None:
        nc = bacc.Bacc(
            "TRN2", target_bir_lowering=False, debug=False, num_devices=NCORES
        )
        with tile.TileContext(nc) as tc:
            _emit(tc)
        nc.compile()
        _NC_CACHE = nc
    return _NC_CACHE


def _in_maps(inputs):
    f32 = lambda x: np.ascontiguousarray(np.asarray(x), dtype=np.float32)
    queries = f32(inputs["queries"])
    keys = f32(inputs["keys"])
    values = f32(inputs["values"])
    shared = {
        "Wq": f32(inputs["Wq"]),
        "Wk": f32(inputs["Wk"]),
        "Wv": f32(inputs["Wv"]),
        "Wo": f32(inputs["Wo"]),
        "wv_score": f32(inputs["wv_score"]),
        "fcW": f32(inputs["fcW"]),
        "fcb": f32(inputs["fcb"]),
    }
    maps = []
    for c in range(NCORES):
        sl = slice(c * NB, (c + 1) * NB)
        maps.append(
            {
                "queries": np.ascontiguousarray(queries[sl]),
                "keys": np.ascontiguousarray(keys[sl]),
                "values": np.ascontiguousarray(values[sl]),
                **shared,
            }
        )
    return maps


def run(inputs, trace=False):
    nc = _get_nc()
    res = run_bass_kernel_spmd(
        nc, _in_maps(inputs), core_ids=list(range(NCORES)), trace=trace
    )
    outp = np.concatenate([res.results[c]["out"] for c in range(NCORES)], axis=0)
    return outp, res.exec_time_ns


def run_sim(inputs):
    import concourse.bass_interp as bass_interp

    nc = _get_nc()
    sim = bass_interp.CoreSim(nc)
    for k, v in _in_maps(inputs)[0].items():
        sim.tensor(k)[:] = v
    sim.simulate()
    return np.array(sim.tensor("out"))


def kernel(**inputs):
    return run(inputs, trace=False)[0]



# revision 40
# speedup vs baseline: 1.0999x; 1.0999x over previous
import numpy as np

import concourse.bacc as bacc
import concourse.bass as bass
import concourse.mybir as mybir
import concourse.tile as tile
from concourse.bass_utils import run_bass_kernel_spmd
from concourse.masks import make_identity

B, Q, KL, D = 32, 8, 2048, 256
H, DH = 8, 32
NCORES = 8
NB = B // NCORES
KC = KL // 128
NG = NB * 2
F32 = mybir.dt.float32
BF16 = mybir.dt.bfloat16
Tanh = mybir.ActivationFunctionType.Tanh
Exp = mybir.ActivationFunctionType.Exp
Alu = mybir.AluOpType

SCHEDULE = [
    "AAAAAAPP",
    "AAAAAPPD",
    "AAAAAAPP",
    "AAAAAPPD",
    "AAAAAAPP",
    "AAAAAAPP",
    "AAAAAAPP",
    "AAAAAAAA",
]


def _emit(tc):
    nc = tc.nc

    queries = nc.dram_tensor("queries", [NB, Q, D], F32, kind="ExternalInput").ap()
    keys = nc.dram_tensor("keys", [NB, KL, D], F32, kind="ExternalInput").ap()
    values = nc.dram_tensor("values", [NB, KL, D], F32, kind="ExternalInput").ap()
    Wq = nc.dram_tensor("Wq", [D, D], F32, kind="ExternalInput").ap()
    Wk = nc.dram_tensor("Wk", [D, D], F32, kind="ExternalInput").ap()
    Wv = nc.dram_tensor("Wv", [D, D], F32, kind="ExternalInput").ap()
    Wo = nc.dram_tensor("Wo", [D, D], F32, kind="ExternalInput").ap()
    wv_score = nc.dram_tensor("wv_score", [DH], F32, kind="ExternalInput").ap()
    fcW = nc.dram_tensor("fcW", [D, Q * D], F32, kind="ExternalInput").ap()
    fcb = nc.dram_tensor("fcb", [D], F32, kind="ExternalInput").ap()
    out = nc.dram_tensor("out", [NB, D], F32, kind="ExternalOutput").ap()

    dram = tc.alloc_tile_pool(name="dram", bufs=1, space="DRAM")
    consts = tc.alloc_tile_pool(name="consts", bufs=1)
    krep_pool = tc.alloc_tile_pool(name="krep_ps", bufs=1, space="PSUM")
    sc_pool = tc.alloc_tile_pool(name="sc_ps", bufs=2, space="PSUM")
    small_ps = tc.alloc_tile_pool(name="small_ps", bufs=1, space="PSUM")
    krepsb_pool = tc.alloc_tile_pool(name="krep_sb", bufs=2)
    tk_pool = tc.alloc_tile_pool(name="tk", bufs=2)
    feat_pool = tc.alloc_tile_pool(name="feat", bufs=6)
    dvetmp_pool = tc.alloc_tile_pool(name="dvetmp", bufs=6)
    soft_pool = tc.alloc_tile_pool(name="soft", bufs=2)
    en_pool = tc.alloc_tile_pool(name="en", bufs=6)
    r_pool = tc.alloc_tile_pool(name="r_ps", bufs=1, space="PSUM")
    r_pool_sb = tc.alloc_tile_pool(name="r_sb", bufs=2)
    pools = [
        r_pool_sb, r_pool, en_pool, soft_pool, dvetmp_pool, feat_pool, tk_pool,
        krepsb_pool, small_ps, sc_pool, krep_pool, consts, dram,
    ]

    id32b = consts.tile([32, 32], BF16, tag="id32b", name="id32b")
    id128f = consts.tile([128, 128], F32, tag="id128f", name="id128f")
    id128b = consts.tile([128, 128], BF16, tag="id128b", name="id128b")
    make_identity(nc, id32b[:])
    make_identity(nc, id128f[:])
    make_identity(nc, id128b[:])
    dummy = consts.tile([1, 2], F32, tag="dummy", name="dummy")
    nc.vector.memset(dummy[:], 0.0)
    nc.scalar.activation(out=dummy[:], in_=dummy[:], func=Tanh)

    S_f32 = consts.tile([128, 4], F32, tag="S_f32", name="S_f32")
    S = consts.tile([128, 4], BF16, tag="S", name="S")
    nc.vector.memset(S_f32[:], 0.0)
    wv_col = wv_score.rearrange("(d one) -> d one", one=1)
    for hh in range(4):
        nc.scalar.dma_start(out=S_f32[hh * 32 : (hh + 1) * 32, hh : hh + 1], in_=wv_col)
    nc.vector.tensor_copy(out=S[:], in_=S_f32[:])

    fcb_sb = consts.tile([NB, D], F32, tag="fcb_sb", name="fcb_sb")
    fcb_b = bass.AP(tensor=fcb.tensor, offset=fcb.offset, ap=[[0, NB], [1, D]])
    nc.scalar.dma_start(out=fcb_sb[:], in_=fcb_b)

    misc_ps = small_ps.tile([128, 448], F32, tag="misc", name="misc_ps")
    o2_r = misc_ps[:, : NB * 2 * Q].rearrange("p (b m q) -> p b m q", b=NB, m=2, q=Q)

    keys_bf = dram.tile([NB, KL, D], BF16)
    chain = nc.gpsimd.dma_start(out=keys_bf[0], in_=keys[0])

    def chain_cast(out_ap, in_ap):
        nonlocal chain
        nxt = nc.gpsimd.dma_start(out=out_ap, in_=in_ap)
        tile.add_dep_helper(nxt.ins, chain.ins, reason="pool dma order")
        chain = nxt
        return nxt



    def wtrans(name, src, n=2):
        ts = [
            consts.tile([128, D], BF16, tag=f"{name}{ch}", name=f"{name}{ch}")
            for ch in range(n)
        ]
        for ch in range(n):
            nc.sync.dma_start(
                out=ts[ch][:], in_=src[:, ch * 128 : (ch + 1) * 128], transpose=True
            )
        return ts

    def pe_wtrans(name, W, queue=None):
        queue = queue or nc.sync
        nat = [
            consts.tile([128, D], F32, tag=f"{name}nat{j}", name=f"{name}nat{j}")
            for j in range(2)
        ]
        for j in range(2):
            queue.dma_start(out=nat[j][:], in_=W[j * 128 : (j + 1) * 128, :])
        ts = [
            consts.tile([128, D], BF16, tag=f"{name}T{ch}", name=f"{name}T{ch}")
            for ch in range(2)
        ]
        for ch in range(2):
            for j in range(2):
                tp = r_pool.tile([128, 128], F32, tag="r_ps", name=f"{name}T_ps")[:]
                nc.tensor.transpose(
                    out=tp,
                    in_=nat[j][:, ch * 128 : (ch + 1) * 128],
                    identity=id128f[:],
                )
                nc.vector.tensor_copy(
                    out=ts[ch][:, j * 128 : (j + 1) * 128], in_=tp
                )
        return ts

    WqT = pe_wtrans("Wq", Wq)
    WkT = pe_wtrans("Wk", Wk)

    q_nat = consts.tile([NB * Q, D], BF16, tag="q_nat", name="q_nat")
    nc.gpsimd.dma_start(out=q_nat[:], in_=queries.rearrange("b q d -> (b q) d"))
    qT = [consts.tile([128, NB * Q], BF16, tag=f"qT{ch}", name=f"qT{ch}") for ch in range(2)]
    for ch in range(2):
        qT_ps = r_pool.tile([128, 128], F32, tag="r_ps", name="qT_ps")
        qT_ps_bf = qT_ps[:, : NB * Q // 2].bitcast(BF16)
        nc.tensor.transpose(
            out=qT_ps_bf, in_=q_nat[:, ch * 128 : (ch + 1) * 128], identity=id32b[:]
        )
        nc.vector.tensor_copy(out=qT[ch][:], in_=qT_ps_bf)
    qpT = [consts.tile([128, NB * Q], F32, tag=f"qpT{hg}", name=f"qpT{hg}") for hg in range(2)]
    TqT = [consts.tile([128, NB * Q], F32, tag=f"TqT{hg}", name=f"TqT{hg}") for hg in range(2)]
    for hg in range(2):
        qpT_ps = r_pool.tile([128, 128], F32, tag="r_ps", name="qpT_ps")
        for ch in range(2):
            nc.tensor.matmul(
                out=qpT_ps[:, : NB * Q],
                lhsT=WqT[ch][:, hg * 128 : (hg + 1) * 128],
                rhs=qT[ch][:],
                start=(ch == 0),
                stop=(ch == 1),
            )
        nc.vector.tensor_copy(out=qpT[hg][:], in_=qpT_ps[:, : NB * Q])
        nc.scalar.activation(out=TqT[hg][:], in_=qpT_ps[:, : NB * Q], func=Tanh)

    keysT_pool = tc.alloc_tile_pool(name="keysT", bufs=6)
    vnat_pool = tc.alloc_tile_pool(name="vnat", bufs=2)
    pools.insert(0, keysT_pool)
    pools.insert(0, vnat_pool)
    keysT = {}
    v_nat = {}
    en_tiles = {}
    r_tiles = {}
    sc_tiles = {}

    def load_keysT(b):
        keysT[b] = [keysT_pool.tile([128, KL], BF16, name="keysT") for ch in range(2)]
        trs = []
        for ch in range(2):
            eng = nc.sync
            trs.append(eng.dma_start(
                out=keysT[b][ch][:],
                in_=keys_bf[b, :, ch * 128 : (ch + 1) * 128],
                transpose=True,
            ))
        return trs

    def load_values(b):
        v_nat[b] = vnat_pool.tile([128, KC * D], BF16, name="vnat")
        chain_cast(
            v_nat[b][:].rearrange("p (kc d) -> p kc d", kc=KC, d=D),
            values[b].rearrange("(kc p) d -> p kc d", p=128),
        )

    def emit_kproj(b, hg):
        krep_ps = krep_pool.tile([128, KL], F32, name="krep_ps")
        for nco in range(4):
            sl = slice(nco * 512, (nco + 1) * 512)
            for ch in range(2):
                nc.tensor.matmul(
                    out=krep_ps[:, sl],
                    lhsT=WkT[ch][:, hg * 128 : (hg + 1) * 128],
                    rhs=keysT[b][ch][:, sl],
                    start=(ch == 0),
                    stop=(ch == 1),
                )
        krep_sb = krepsb_pool.tile([128, KL], BF16, name="krep_sb")
        nc.vector.tensor_copy(out=krep_sb[:], in_=krep_ps[:])
        return krep_sb

    def emit_feats(g, b, hg, krep_sb):
        sched = SCHEDULE[g]
        qcol = lambda q: qpT[hg][:, b * Q + q : b * Q + q + 1]
        tqcol = lambda q: TqT[hg][:, b * Q + q : b * Q + q + 1]

        tk = None
        if any(c != "A" for c in sched):
            tk = tk_pool.tile([128, KL], BF16, name="tk")
            nc.scalar.activation(out=tk[:], in_=krep_sb[:], func=Tanh)

        sc_ps = sc_pool.tile([128, 512], F32, name="sc_ps")
        sc_r = sc_ps[:].rearrange("p (kc q h) -> p kc q h", kc=KC, q=Q, h=4)
        qorder = [q for q in range(Q) if sched[q] == "A"] + \
                 [q for q in range(Q) if sched[q] == "D"] + \
                 [q for q in range(Q) if sched[q] == "P"]
        for q in qorder:
            path = sched[q]
            feat = feat_pool.tile([128, KL], BF16, name="feat")
            if path == "A":
                nc.scalar.activation(
                    out=feat[:], in_=krep_sb[:], func=Tanh, bias=qcol(q)
                )
            else:
                eng = nc.gpsimd if path == "P" else nc.vector
                den = dvetmp_pool.tile([128, KL], BF16, name="den")
                eng.tensor_scalar(
                    out=den[:], in0=tk[:], scalar1=tqcol(q), scalar2=1.0,
                    op0=Alu.mult, op1=Alu.add,
                )
                rec = dvetmp_pool.tile([128, KL], BF16, name="rec")
                with nc.allow_low_precision(reason="bf16 reciprocal: tanh identity denominator in [0.1, 1.9]"):
                    nc.vector.reciprocal(out=rec[:], in_=den[:])
                num = dvetmp_pool.tile([128, KL], BF16, name="num")
                eng.tensor_scalar(
                    out=num[:], in0=tk[:], scalar1=tqcol(q), scalar2=None,
                    op0=Alu.add,
                )
                nc.vector.tensor_tensor(out=feat[:], in0=num[:], in1=rec[:], op=Alu.mult)
            for kc in range(KC):
                nc.tensor.matmul(
                    out=sc_r[:, kc, q, :],
                    lhsT=feat[:, kc * 128 : (kc + 1) * 128],
                    rhs=S[:],
                    start=True,
                    stop=True,
                )
        sc_tiles[g] = sc_ps

    def emit_softmax(g, b, hg):
        sc_ps = sc_tiles.pop(g)
        exp_sb = soft_pool.tile([128, 512], BF16, tag="exp_sb", name="exp_sb")
        nc.scalar.activation(out=exp_sb[:], in_=sc_ps[:], func=Exp)
        Zt = soft_pool.tile([128, 64], F32, tag="Zt", name="Zt")
        exp_khq = exp_sb[:].rearrange("p (kc q h) -> p kc h q", kc=KC, q=Q, h=4)
        nc.vector.tensor_reduce(
            out=Zt[:], in_=exp_khq, axis=mybir.AxisListType.X, op=Alu.add
        )
        invZ = soft_pool.tile([128, 64], BF16, tag="invZ", name="invZ")
        with nc.allow_low_precision(reason="softmax 1/Z in bf16; Z in [8e-1, 10]"):
            nc.vector.reciprocal(out=invZ[:], in_=Zt[:])
        en = en_pool.tile([128, 512], BF16, name="en")
        in0 = exp_sb[:].rearrange("p (kc q h) -> p kc q h", kc=KC, q=Q, h=4)
        iz = invZ[:].rearrange("p (kc h) -> p kc h", kc=KC, h=4)
        in1 = bass.AP(
            tensor=iz.tensor,
            offset=iz.offset,
            ap=[list(iz.ap[0]), list(iz.ap[1]), [0, Q], list(iz.ap[2])],
        )
        en_r = en[:].rearrange("p (kc q h) -> p kc q h", kc=KC, q=Q, h=4)
        nc.vector.tensor_tensor(out=en_r, in0=in0, in1=in1, op=Alu.mult)
        en_tiles[(b, hg)] = en

    def emit_R(b):
        r_ps = r_pool.tile([128, 2 * Q * H], F32, name="r_ps")
        r_r = r_ps[:].rearrange("p (c g q h) -> p c g q h", c=2, g=2, q=Q, h=4)
        for hg in range(2):
            en_r = en_tiles[(b, hg)][:].rearrange(
                "p (kc q h) -> p kc q h", kc=KC, q=Q, h=4
            )
            for ch in range(2):
                for kc in range(KC):
                    nc.tensor.matmul(
                        out=r_r[:, ch, hg, :, :],
                        lhsT=v_nat[b][:, kc * D + ch * 128 : kc * D + (ch + 1) * 128],
                        rhs=en_r[:, kc, :, :],
                        start=(kc == 0),
                        stop=(kc == KC - 1),
                    )
        r_sb = r_pool_sb.tile([128, 2 * Q * H], BF16, name="r_sb")
        nc.vector.tensor_copy(out=r_sb[:], in_=r_ps[:])
        r_tiles[b] = r_sb


    def emit_out2T(b):
        r_r = r_tiles[b][:].rearrange("p (c g q h) -> p c g q h", c=2, g=2, q=Q, h=4)
        for m in range(2):
            first = True
            for ch in range(2):
                for h in range(H):
                    nc.tensor.matmul(
                        out=o2_r[:, b, m, :],
                        lhsT=W2sb[ch][:, h * D + m * 128 : h * D + (m + 1) * 128],
                        rhs=r_r[:, ch, h // 4, :, h % 4],
                        start=first,
                        stop=(ch == 1 and h == H - 1),
                    )
                    first = False


    k0_trs = load_keysT(0)
    keys1_cast = chain_cast(keys_bf[1], keys[1])
    for tr in k0_trs:
        tile.add_dep_helper(keys1_cast.ins, tr.ins, reason="keys0 transposes first")
    krep_g = {0: emit_kproj(0, 0)}
    fcwT = []
    load_keysT(1)

    load_values(0)
    wv_natf = [
        consts.tile([128, D], F32, tag=f"Wvnatf{j}", name=f"Wvnatf{j}")
        for j in range(2)
    ]
    for j in range(2):
        nc.sync.dma_start(out=wv_natf[j][:], in_=Wv[j * 128 : (j + 1) * 128, :])
    Wv_nat = [consts.tile([128, D], BF16, tag=f"Wvn{ch}", name=f"Wvn{ch}") for ch in range(2)]
    for ch in range(2):
        nc.vector.tensor_copy(out=Wv_nat[ch][:], in_=wv_natf[ch][:])
    WoT = pe_wtrans("Wo", Wo)
    Wv_h, WoT_h = {}, {}
    for h in range(H):
        ch, off = h // 4, (h % 4) * 32
        if off == 96:
            rebv = consts.tile([32, D], BF16, tag=f"Wvh{h}", name=f"Wvh{h}")
            rebo = consts.tile([32, D], BF16, tag=f"WoTh{h}", name=f"WoTh{h}")
            nc.sync.dma_start(out=rebv[:], in_=Wv_nat[ch][off : off + 32, :])
            nc.sync.dma_start(out=rebo[:], in_=WoT[ch][off : off + 32, :])
            Wv_h[h], WoT_h[h] = rebv[:], rebo[:]
        else:
            Wv_h[h] = Wv_nat[ch][off : off + 32, :]
            WoT_h[h] = WoT[ch][off : off + 32, :]
    W2sb = [
        consts.tile([128, H * D], BF16, tag=f"W2sb{m}", name=f"W2sb{m}")
        for m in range(2)
    ]

    def w2_piece(m, h, oh):
        w2_ps = r_pool.tile([128, 128], F32, tag="r_ps", name="w2_ps")[:]
        nc.tensor.matmul(
            out=w2_ps,
            lhsT=Wv_h[h][:, m * 128 : (m + 1) * 128],
            rhs=WoT_h[h][:, oh * 128 : (oh + 1) * 128],
            start=True,
            stop=True,
        )
        nc.vector.tensor_copy(
            out=W2sb[m][:, h * D + oh * 128 : h * D + (oh + 1) * 128], in_=w2_ps
        )

    w2_pieces = [(m, h, oh) for m in range(2) for h in range(H) for oh in range(2)]

    for g in range(NG):
        b, hg = divmod(g, 2)
        if g + 1 < NG:
            b2, hg2 = divmod(g + 1, 2)
            krep_g[g + 1] = emit_kproj(b2, hg2)
        if hg == 0:
            if b + 2 < NB:
                chain_cast(keys_bf[b + 2], keys[b + 2])
            if b + 1 < NB:
                load_values(b + 1)
            if b == 0:
                fcw_nat = [
                    consts.tile([128, Q * D], BF16, tag=f"fcwnat{ch}", name=f"fcwnat{ch}")
                    for ch in range(2)
                ]
                for ch in range(2):
                    chain_cast(fcw_nat[ch][:], fcW[ch * 128 : (ch + 1) * 128, :])
        emit_feats(g, b, hg, krep_g.pop(g))
        if g >= 1:
            pb, phg = divmod(g - 1, 2)
            emit_softmax(g - 1, pb, phg)
            if phg == 1:
                emit_R(pb)
                emit_out2T(pb)
        if hg == 1 and b + 2 < NB:
            load_keysT(b + 2)
        if g <= 1:
            for (m, h, oh) in w2_pieces[g * 16 : (g + 1) * 16]:
                w2_piece(m, h, oh)
        if 1 <= g <= 4:
            for t in range((g - 1) * 4, g * 4):
                tile_bf = consts.tile([128, D], BF16, tag=f"fcwT{t}", name=f"fcwT{t}")
                for ch in range(2):
                    tp = misc_ps[:, 320 : 320 + 64].bitcast(BF16)
                    nc.tensor.transpose(
                        out=tp,
                        in_=fcw_nat[ch][:, t * 128 : (t + 1) * 128],
                        identity=id128b[:],
                    )
                    nc.vector.tensor_copy(
                        out=tile_bf[:, ch * 128 : (ch + 1) * 128], in_=tp
                    )
                fcwT.append(tile_bf)
    emit_softmax(NG - 1, NB - 1, 1)
    emit_R(NB - 1)
    emit_out2T(NB - 1)

    o2sb = consts.tile([128, NB * 2 * Q], BF16, tag="o2sb", name="o2sb")
    nc.vector.tensor_copy(out=o2sb[:], in_=misc_ps[:, : NB * 2 * Q])
    o2sb_r = o2sb[:].rearrange("p (b m q) -> p m q b", b=NB, m=2, q=Q)
    for t in range(16):
        qq, m = t // 2, t % 2
        nc.tensor.matmul(
            out=misc_ps[0:NB, 64 : 64 + D],
            lhsT=o2sb_r[:, m, qq, :],
            rhs=fcwT[t][:],
            start=(t == 0),
            stop=(t == 15),
        )
    y_sb = consts.tile([NB, D], F32, tag="y_sb", name="y_sb")
    nc.vector.tensor_tensor(
        out=y_sb[:], in0=misc_ps[0:NB, 64 : 64 + D], in1=fcb_sb[:], op=Alu.add
    )
    nc.sync.dma_start(out=out, in_=y_sb[:])

    for p in pools:
        p.release()


_NC_CACHE = None


def _get_nc():
    global _NC_CACHE
    if _NC_CACHE is None:
        nc = bacc.Bacc(
            "TRN2", target_bir_lowering=False, debug=False, num_devices=NCORES
        )
        with tile.TileContext(nc) as tc:
            _emit(tc)
        nc.compile()
        _NC_CACHE = nc
    return _NC_CACHE


def _in_maps(inputs):
    f32 = lambda x: np.ascontiguousarray(np.asarray(x), dtype=np.float32)
    queries = f32(inputs["queries"])
    keys = f32(inputs["keys"])
    values = f32(inputs["values"])
    shared = {
        "Wq": f32(inputs["Wq"]),
        "Wk": f32(inputs["Wk"]),
        "Wv": f32(inputs["Wv"]),
        "Wo": f32(inputs["Wo"]),
        "wv_score": f32(inputs["wv_score"]),
        "fcW": f32(inputs["fcW"]),
        "fcb": f32(inputs["fcb"]),
    }
    maps = []
    for c in range(NCORES):
        sl = slice(c * NB, (c + 1) * NB)
        maps.append(
            {
                "queries": np.ascontiguousarray(queries[sl]),
                "keys": np.ascontiguousarray(keys[sl]),
                "values": np.ascontiguousarray(values[sl]),
                **shared,
            }
        )
    return maps


def run(inputs, trace=False):
    nc = _get_nc()
    res = run_bass_kernel_spmd(
        nc, _in_maps(inputs), core_ids=list(range(NCORES)), trace=trace
    )
    outp = np.concatenate([res.results[c]["out"] for c in range(NCORES)], axis=0)
    return outp, res.exec_time_ns


def run_sim(inputs):
    import concourse.bass_interp as bass_interp

    nc = _get_nc()
    sim = bass_interp.CoreSim(nc)
    for k, v in _in_maps(inputs)[0].items():
        sim.tensor(k)[:] = v
    sim.simulate()
    return np.array(sim.tensor("out"))


def kernel(**inputs):
    return run(inputs, trace=False)[0]


# revision 41
# speedup vs baseline: 1.2004x; 1.0914x over previous
import numpy as np

import concourse.bacc as bacc
import concourse.bass as bass
import concourse.mybir as mybir
import concourse.tile as tile
from concourse.bass_utils import run_bass_kernel_spmd
from concourse.masks import make_identity

B, Q, KL, D = 32, 8, 2048, 256
H, DH = 8, 32
NCORES = 8
NB = B // NCORES
KC = KL // 128
NG = NB * 2
F32 = mybir.dt.float32
BF16 = mybir.dt.bfloat16
Tanh = mybir.ActivationFunctionType.Tanh
Exp = mybir.ActivationFunctionType.Exp
Alu = mybir.AluOpType

SCHEDULE = [
    "AAAAAAPP",
    "AAAAAAPD",
    "AAAAAAPP",
    "AAAAAAPD",
    "AAAAAAPP",
    "AAAAAAPP",
    "AAAAAAPP",
    "AAAAAAAA",
]


def _emit(tc):
    nc = tc.nc

    queries = nc.dram_tensor("queries", [NB, Q, D], F32, kind="ExternalInput").ap()
    keys = nc.dram_tensor("keys", [NB, KL, D], F32, kind="ExternalInput").ap()
    values = nc.dram_tensor("values", [NB, KL, D], F32, kind="ExternalInput").ap()
    Wq = nc.dram_tensor("Wq", [D, D], F32, kind="ExternalInput").ap()
    Wk = nc.dram_tensor("Wk", [D, D], F32, kind="ExternalInput").ap()
    Wv = nc.dram_tensor("Wv", [D, D], F32, kind="ExternalInput").ap()
    Wo = nc.dram_tensor("Wo", [D, D], F32, kind="ExternalInput").ap()
    wv_score = nc.dram_tensor("wv_score", [DH], F32, kind="ExternalInput").ap()
    fcW = nc.dram_tensor("fcW", [D, Q * D], F32, kind="ExternalInput").ap()
    fcb = nc.dram_tensor("fcb", [D], F32, kind="ExternalInput").ap()
    out = nc.dram_tensor("out", [NB, D], F32, kind="ExternalOutput").ap()

    dram = tc.alloc_tile_pool(name="dram", bufs=1, space="DRAM")
    consts = tc.alloc_tile_pool(name="consts", bufs=1)
    krep_pool = tc.alloc_tile_pool(name="krep_ps", bufs=1, space="PSUM")
    sc_pool = tc.alloc_tile_pool(name="sc_ps", bufs=2, space="PSUM")
    small_ps = tc.alloc_tile_pool(name="small_ps", bufs=1, space="PSUM")
    krepsb_pool = tc.alloc_tile_pool(name="krep_sb", bufs=2)
    ref_pool = tc.alloc_tile_pool(name="ref", bufs=2)
    feat_pool = tc.alloc_tile_pool(name="feat", bufs=6)
    dvetmp_pool = tc.alloc_tile_pool(name="dvetmp", bufs=6)
    soft_pool = tc.alloc_tile_pool(name="soft", bufs=2)
    en_pool = tc.alloc_tile_pool(name="en", bufs=6)
    r_pool = tc.alloc_tile_pool(name="r_ps", bufs=1, space="PSUM")
    r_pool_sb = tc.alloc_tile_pool(name="r_sb", bufs=2)
    pools = [
        r_pool_sb, r_pool, en_pool, soft_pool, dvetmp_pool, feat_pool, ref_pool,
        krepsb_pool, small_ps, sc_pool, krep_pool, consts, dram,
    ]

    id32b = consts.tile([32, 32], BF16, tag="id32b", name="id32b")
    id128f = consts.tile([128, 128], F32, tag="id128f", name="id128f")
    id128b = consts.tile([128, 128], BF16, tag="id128b", name="id128b")
    make_identity(nc, id32b[:])
    make_identity(nc, id128f[:])
    make_identity(nc, id128b[:])
    dummy = consts.tile([1, 2], F32, tag="dummy", name="dummy")
    nc.vector.memset(dummy[:], 0.0)
    nc.scalar.activation(out=dummy[:], in_=dummy[:], func=Tanh)

    S_f32 = consts.tile([128, 4], F32, tag="S_f32", name="S_f32")
    S = consts.tile([128, 4], BF16, tag="S", name="S")
    nc.vector.memset(S_f32[:], 0.0)
    wv_col = wv_score.rearrange("(d one) -> d one", one=1)
    for hh in range(4):
        nc.scalar.dma_start(out=S_f32[hh * 32 : (hh + 1) * 32, hh : hh + 1], in_=wv_col)
    nc.vector.tensor_copy(out=S[:], in_=S_f32[:])

    fcb_sb = consts.tile([NB, D], F32, tag="fcb_sb", name="fcb_sb")
    fcb_b = bass.AP(tensor=fcb.tensor, offset=fcb.offset, ap=[[0, NB], [1, D]])
    nc.scalar.dma_start(out=fcb_sb[:], in_=fcb_b)

    misc_ps = small_ps.tile([128, 448], F32, tag="misc", name="misc_ps")
    o2_r = misc_ps[:, : NB * 2 * Q].rearrange("p (b m q) -> p b m q", b=NB, m=2, q=Q)

    keys_bf = dram.tile([NB, KL, D], BF16)
    chain = nc.gpsimd.dma_start(out=keys_bf[0], in_=keys[0])

    def chain_cast(out_ap, in_ap):
        nonlocal chain
        nxt = nc.gpsimd.dma_start(out=out_ap, in_=in_ap)
        tile.add_dep_helper(nxt.ins, chain.ins, reason="pool dma order")
        chain = nxt
        return nxt



    def wtrans(name, src, n=2):
        ts = [
            consts.tile([128, D], BF16, tag=f"{name}{ch}", name=f"{name}{ch}")
            for ch in range(n)
        ]
        for ch in range(n):
            nc.sync.dma_start(
                out=ts[ch][:], in_=src[:, ch * 128 : (ch + 1) * 128], transpose=True
            )
        return ts

    def pe_wtrans(name, W, queue=None):
        queue = queue or nc.sync
        nat = [
            consts.tile([128, D], F32, tag=f"{name}nat{j}", name=f"{name}nat{j}")
            for j in range(2)
        ]
        for j in range(2):
            queue.dma_start(out=nat[j][:], in_=W[j * 128 : (j + 1) * 128, :])
        ts = [
            consts.tile([128, D], BF16, tag=f"{name}T{ch}", name=f"{name}T{ch}")
            for ch in range(2)
        ]
        for ch in range(2):
            for j in range(2):
                tp = r_pool.tile([128, 128], F32, tag="r_ps", name=f"{name}T_ps")[:]
                nc.tensor.transpose(
                    out=tp,
                    in_=nat[j][:, ch * 128 : (ch + 1) * 128],
                    identity=id128f[:],
                )
                nc.vector.tensor_copy(
                    out=ts[ch][:, j * 128 : (j + 1) * 128], in_=tp
                )
        return ts

    WqT = pe_wtrans("Wq", Wq)
    WkT = pe_wtrans("Wk", Wk)

    q_nat = consts.tile([NB * Q, D], BF16, tag="q_nat", name="q_nat")
    nc.gpsimd.dma_start(out=q_nat[:], in_=queries.rearrange("b q d -> (b q) d"))
    qT = [consts.tile([128, NB * Q], BF16, tag=f"qT{ch}", name=f"qT{ch}") for ch in range(2)]
    for ch in range(2):
        qT_ps = r_pool.tile([128, 128], F32, tag="r_ps", name="qT_ps")
        qT_ps_bf = qT_ps[:, : NB * Q // 2].bitcast(BF16)
        nc.tensor.transpose(
            out=qT_ps_bf, in_=q_nat[:, ch * 128 : (ch + 1) * 128], identity=id32b[:]
        )
        nc.vector.tensor_copy(out=qT[ch][:], in_=qT_ps_bf)
    qpT = [consts.tile([128, NB * Q], F32, tag=f"qpT{hg}", name=f"qpT{hg}") for hg in range(2)]
    TqT = [consts.tile([128, NB * Q], F32, tag=f"TqT{hg}", name=f"TqT{hg}") for hg in range(2)]
    for hg in range(2):
        qpT_ps = r_pool.tile([128, 128], F32, tag="r_ps", name="qpT_ps")
        for ch in range(2):
            nc.tensor.matmul(
                out=qpT_ps[:, : NB * Q],
                lhsT=WqT[ch][:, hg * 128 : (hg + 1) * 128],
                rhs=qT[ch][:],
                start=(ch == 0),
                stop=(ch == 1),
            )
        nc.vector.tensor_copy(out=qpT[hg][:], in_=qpT_ps[:, : NB * Q])
        nc.scalar.activation(out=TqT[hg][:], in_=qpT_ps[:, : NB * Q], func=Tanh)

    keysT_pool = tc.alloc_tile_pool(name="keysT", bufs=6)
    vnat_pool = tc.alloc_tile_pool(name="vnat", bufs=2)
    pools.insert(0, keysT_pool)
    pools.insert(0, vnat_pool)
    keysT = {}
    v_nat = {}
    en_tiles = {}
    r_tiles = {}
    sc_tiles = {}

    def load_keysT(b):
        keysT[b] = [keysT_pool.tile([128, KL], BF16, name="keysT") for ch in range(2)]
        trs = []
        for ch in range(2):
            eng = nc.sync
            trs.append(eng.dma_start(
                out=keysT[b][ch][:],
                in_=keys_bf[b, :, ch * 128 : (ch + 1) * 128],
                transpose=True,
            ))
        return trs

    def load_values(b):
        v_nat[b] = vnat_pool.tile([128, KC * D], BF16, name="vnat")
        chain_cast(
            v_nat[b][:].rearrange("p (kc d) -> p kc d", kc=KC, d=D),
            values[b].rearrange("(kc p) d -> p kc d", p=128),
        )

    def emit_kproj(b, hg):
        krep_ps = krep_pool.tile([128, KL], F32, name="krep_ps")
        for nco in range(4):
            sl = slice(nco * 512, (nco + 1) * 512)
            for ch in range(2):
                nc.tensor.matmul(
                    out=krep_ps[:, sl],
                    lhsT=WkT[ch][:, hg * 128 : (hg + 1) * 128],
                    rhs=keysT[b][ch][:, sl],
                    start=(ch == 0),
                    stop=(ch == 1),
                )
        krep_sb = krepsb_pool.tile([128, KL], BF16, name="krep_sb")
        nc.vector.tensor_copy(out=krep_sb[:], in_=krep_ps[:])
        return krep_sb

    def emit_feats(g, b, hg, krep_sb):
        sched = SCHEDULE[g]
        qcol = lambda q: qpT[hg][:, b * Q + q : b * Q + q + 1]

        qorder = [q for q in range(Q) if sched[q] == "A"] + \
                 [q for q in range(Q) if sched[q] == "D"] + \
                 [q for q in range(Q) if sched[q] == "P"]
        idqs = [q for q in qorder if sched[q] != "A"]
        q_ref = qorder[0] if idqs else None

        tdcol = {}
        if idqs:
            qp_id = bass.AP(
                tensor=qpT[hg][:].tensor,
                offset=qpT[hg][:].offset + b * Q,
                ap=[list(qpT[hg][:].ap[0])] + [[1, 0]],
            )
            td = consts.tile([128, Q], F32, tag=f"td{g}", name=f"td{g}")
            dif = consts.tile([128, Q], F32, tag=f"dif{g}", name=f"dif{g}")
            refb = bass.AP(
                tensor=qpT[hg][:].tensor,
                offset=qpT[hg][:].offset,
                ap=[list(qpT[hg][:].ap[0]), [0, Q]],
            )
            refb.offset += b * Q + q_ref
            nc.vector.tensor_tensor(
                out=dif[:],
                in0=qpT[hg][:, b * Q : b * Q + Q],
                in1=refb,
                op=Alu.subtract,
            )
            nc.scalar.activation(out=td[:], in_=dif[:], func=Tanh)
            for q in idqs:
                tdcol[q] = td[:, q : q + 1]

        sc_ps = sc_pool.tile([128, 512], F32, name="sc_ps")
        sc_r = sc_ps[:].rearrange("p (kc q h) -> p kc q h", kc=KC, q=Q, h=4)
        ref_feat = None
        for q in qorder:
            path = sched[q]
            if path == "A":
                if q == q_ref and idqs:
                    feat = ref_pool.tile([128, KL], BF16, name="ref_feat")
                else:
                    feat = feat_pool.tile([128, KL], BF16, name="feat")
                nc.scalar.activation(
                    out=feat[:], in_=krep_sb[:], func=Tanh, bias=qcol(q)
                )
                if q == q_ref:
                    ref_feat = feat
            else:
                eng = nc.gpsimd if path == "P" else nc.vector
                feat = feat_pool.tile([128, KL], BF16, name="feat")
                den = dvetmp_pool.tile([128, KL], BF16, name="den")
                eng.tensor_scalar(
                    out=den[:], in0=ref_feat[:], scalar1=tdcol[q], scalar2=1.0,
                    op0=Alu.mult, op1=Alu.add,
                )
                rec = dvetmp_pool.tile([128, KL], BF16, name="rec")
                with nc.allow_low_precision(reason="bf16 reciprocal: tanh identity denominator in [0.1, 1.9]"):
                    nc.vector.reciprocal(out=rec[:], in_=den[:])
                num = dvetmp_pool.tile([128, KL], BF16, name="num")
                eng.tensor_scalar(
                    out=num[:], in0=ref_feat[:], scalar1=tdcol[q], scalar2=None,
                    op0=Alu.add,
                )
                nc.vector.tensor_tensor(out=feat[:], in0=num[:], in1=rec[:], op=Alu.mult)
            for kc in range(KC):
                nc.tensor.matmul(
                    out=sc_r[:, kc, q, :],
                    lhsT=feat[:, kc * 128 : (kc + 1) * 128],
                    rhs=S[:],
                    start=True,
                    stop=True,
                )
        sc_tiles[g] = sc_ps

    def emit_softmax(g, b, hg):
        sc_ps = sc_tiles.pop(g)
        exp_sb = soft_pool.tile([128, 512], BF16, tag="exp_sb", name="exp_sb")
        nc.scalar.activation(out=exp_sb[:], in_=sc_ps[:], func=Exp)
        Zt = soft_pool.tile([128, 64], F32, tag="Zt", name="Zt")
        exp_khq = exp_sb[:].rearrange("p (kc q h) -> p kc h q", kc=KC, q=Q, h=4)
        nc.vector.tensor_reduce(
            out=Zt[:], in_=exp_khq, axis=mybir.AxisListType.X, op=Alu.add
        )
        invZ = soft_pool.tile([128, 64], BF16, tag="invZ", name="invZ")
        with nc.allow_low_precision(reason="softmax 1/Z in bf16; Z in [8e-1, 10]"):
            nc.vector.reciprocal(out=invZ[:], in_=Zt[:])
        en = en_pool.tile([128, 512], BF16, name="en")
        in0 = exp_sb[:].rearrange("p (kc q h) -> p kc q h", kc=KC, q=Q, h=4)
        iz = invZ[:].rearrange("p (kc h) -> p kc h", kc=KC, h=4)
        in1 = bass.AP(
            tensor=iz.tensor,
            offset=iz.offset,
            ap=[list(iz.ap[0]), list(iz.ap[1]), [0, Q], list(iz.ap[2])],
        )
        en_r = en[:].rearrange("p (kc q h) -> p kc q h", kc=KC, q=Q, h=4)
        nc.vector.tensor_tensor(out=en_r, in0=in0, in1=in1, op=Alu.mult)
        en_tiles[(b, hg)] = en

    def emit_R(b):
        r_ps = r_pool.tile([128, 2 * Q * H], F32, name="r_ps")
        r_r = r_ps[:].rearrange("p (c g q h) -> p c g q h", c=2, g=2, q=Q, h=4)
        for hg in range(2):
            en_r = en_tiles[(b, hg)][:].rearrange(
                "p (kc q h) -> p kc q h", kc=KC, q=Q, h=4
            )
            for ch in range(2):
                for kc in range(KC):
                    nc.tensor.matmul(
                        out=r_r[:, ch, hg, :, :],
                        lhsT=v_nat[b][:, kc * D + ch * 128 : kc * D + (ch + 1) * 128],
                        rhs=en_r[:, kc, :, :],
                        start=(kc == 0),
                        stop=(kc == KC - 1),
                    )
        r_sb = r_pool_sb.tile([128, 2 * Q * H], BF16, name="r_sb")
        nc.vector.tensor_copy(out=r_sb[:], in_=r_ps[:])
        r_tiles[b] = r_sb


    def emit_out2T(b):
        r_r = r_tiles[b][:].rearrange("p (c g q h) -> p c g q h", c=2, g=2, q=Q, h=4)
        for m in range(2):
            first = True
            for ch in range(2):
                for h in range(H):
                    nc.tensor.matmul(
                        out=o2_r[:, b, m, :],
                        lhsT=W2sb[ch][:, h * D + m * 128 : h * D + (m + 1) * 128],
                        rhs=r_r[:, ch, h // 4, :, h % 4],
                        start=first,
                        stop=(ch == 1 and h == H - 1),
                    )
                    first = False


    k0_trs = load_keysT(0)
    keys1_cast = chain_cast(keys_bf[1], keys[1])
    for tr in k0_trs:
        tile.add_dep_helper(keys1_cast.ins, tr.ins, reason="keys0 transposes first")
    krep_g = {0: emit_kproj(0, 0)}
    fcwT = []
    load_keysT(1)

    load_values(0)
    wv_natf = [
        consts.tile([128, D], F32, tag=f"Wvnatf{j}", name=f"Wvnatf{j}")
        for j in range(2)
    ]
    for j in range(2):
        nc.sync.dma_start(out=wv_natf[j][:], in_=Wv[j * 128 : (j + 1) * 128, :])
    Wv_nat = [consts.tile([128, D], BF16, tag=f"Wvn{ch}", name=f"Wvn{ch}") for ch in range(2)]
    for ch in range(2):
        nc.vector.tensor_copy(out=Wv_nat[ch][:], in_=wv_natf[ch][:])
    WoT = pe_wtrans("Wo", Wo)
    Wv_h, WoT_h = {}, {}
    for h in range(H):
        ch, off = h // 4, (h % 4) * 32
        if off == 96:
            rebv = consts.tile([32, D], BF16, tag=f"Wvh{h}", name=f"Wvh{h}")
            rebo = consts.tile([32, D], BF16, tag=f"WoTh{h}", name=f"WoTh{h}")
            nc.sync.dma_start(out=rebv[:], in_=Wv_nat[ch][off : off + 32, :])
            nc.sync.dma_start(out=rebo[:], in_=WoT[ch][off : off + 32, :])
            Wv_h[h], WoT_h[h] = rebv[:], rebo[:]
        else:
            Wv_h[h] = Wv_nat[ch][off : off + 32, :]
            WoT_h[h] = WoT[ch][off : off + 32, :]
    W2sb = [
        consts.tile([128, H * D], BF16, tag=f"W2sb{m}", name=f"W2sb{m}")
        for m in range(2)
    ]

    def w2_piece(m, h, oh):
        w2_ps = r_pool.tile([128, 128], F32, tag="r_ps", name="w2_ps")[:]
        nc.tensor.matmul(
            out=w2_ps,
            lhsT=Wv_h[h][:, m * 128 : (m + 1) * 128],
            rhs=WoT_h[h][:, oh * 128 : (oh + 1) * 128],
            start=True,
            stop=True,
        )
        nc.vector.tensor_copy(
            out=W2sb[m][:, h * D + oh * 128 : h * D + (oh + 1) * 128], in_=w2_ps
        )

    w2_pieces = [(m, h, oh) for m in range(2) for h in range(H) for oh in range(2)]

    for g in range(NG):
        b, hg = divmod(g, 2)
        if g + 1 < NG:
            b2, hg2 = divmod(g + 1, 2)
            krep_g[g + 1] = emit_kproj(b2, hg2)
        if hg == 0:
            if b + 2 < NB:
                chain_cast(keys_bf[b + 2], keys[b + 2])
            if b + 1 < NB:
                load_values(b + 1)
            if b == 0:
                fcw_nat = [
                    consts.tile([128, Q * D], BF16, tag=f"fcwnat{ch}", name=f"fcwnat{ch}")
                    for ch in range(2)
                ]
                for ch in range(2):
                    chain_cast(fcw_nat[ch][:], fcW[ch * 128 : (ch + 1) * 128, :])
        emit_feats(g, b, hg, krep_g.pop(g))
        if g >= 1:
            pb, phg = divmod(g - 1, 2)
            emit_softmax(g - 1, pb, phg)
            if phg == 1:
                emit_R(pb)
                emit_out2T(pb)
        if hg == 1 and b + 2 < NB:
            load_keysT(b + 2)
        if g <= 1:
            for (m, h, oh) in w2_pieces[g * 16 : (g + 1) * 16]:
                w2_piece(m, h, oh)
        if 1 <= g <= 4:
            for t in range((g - 1) * 4, g * 4):
                tile_bf = consts.tile([128, D], BF16, tag=f"fcwT{t}", name=f"fcwT{t}")
                for ch in range(2):
                    tp = misc_ps[:, 320 : 320 + 64].bitcast(BF16)
                    nc.tensor.transpose(
                        out=tp,
                        in_=fcw_nat[ch][:, t * 128 : (t + 1) * 128],
                        identity=id128b[:],
                    )
                    nc.vector.tensor_copy(
                        out=tile_bf[:, ch * 128 : (ch + 1) * 128], in_=tp
                    )
                fcwT.append(tile_bf)
    emit_softmax(NG - 1, NB - 1, 1)
    emit_R(NB - 1)
    emit_out2T(NB - 1)

    o2sb = consts.tile([128, NB * 2 * Q], BF16, tag="o2sb", name="o2sb")
    nc.vector.tensor_copy(out=o2sb[:], in_=misc_ps[:, : NB * 2 * Q])
    o2sb_r = o2sb[:].rearrange("p (b m q) -> p m q b", b=NB, m=2, q=Q)
    for t in range(16):
        qq, m = t // 2, t % 2
        nc.tensor.matmul(
            out=misc_ps[0:NB, 64 : 64 + D],
            lhsT=o2sb_r[:, m, qq, :],
            rhs=fcwT[t][:],
            start=(t == 0),
            stop=(t == 15),
        )
    y_sb = consts.tile([NB, D], F32, tag="y_sb", name="y_sb")
    nc.vector.tensor_tensor(
        out=y_sb[:], in0=misc_ps[0:NB, 64 : 64 + D], in1=fcb_sb[:], op=Alu.add
    )
    nc.sync.dma_start(out=out, in_=y_sb[:])

    for p in pools:
        p.release()


_NC_CACHE = None


def _get_nc():
    global _NC_CACHE
    if _NC_CACHE is None:
        nc = bacc.Bacc(
            "TRN2", target_bir_lowering=False, debug=False, num_devices=NCORES
        )
        with tile.TileContext(nc) as tc:
            _emit(tc)
        nc.compile()
        _NC_CACHE = nc
    return _NC_CACHE


def _in_maps(inputs):
    f32 = lambda x: np.ascontiguousarray(np.asarray(x), dtype=np.float32)
    queries = f32(inputs["queries"])
    keys = f32(inputs["keys"])
    values = f32(inputs["values"])
    shared = {
        "Wq": f32(inputs["Wq"]),
        "Wk": f32(inputs["Wk"]),
        "Wv": f32(inputs["Wv"]),
        "Wo": f32(inputs["Wo"]),
        "wv_score": f32(inputs["wv_score"]),
        "fcW": f32(inputs["fcW"]),
        "fcb": f32(inputs["fcb"]),
    }
    maps = []
    for c in range(NCORES):
        sl = slice(c * NB, (c + 1) * NB)
        maps.append(
            {
                "queries": np.ascontiguousarray(queries[sl]),
                "keys": np.ascontiguousarray(keys[sl]),
                "values": np.ascontiguousarray(values[sl]),
                **shared,
            }
        )
    return maps


def run(inputs, trace=False):
    nc = _get_nc()
    res = run_bass_kernel_spmd(
        nc, _in_maps(inputs), core_ids=list(range(NCORES)), trace=trace
    )
    outp = np.concatenate([res.results[c]["out"] for c in range(NCORES)], axis=0)
    return outp, res.exec_time_ns


def run_sim(inputs):
    import concourse.bass_interp as bass_interp

    nc = _get_nc()
    sim = bass_interp.CoreSim(nc)
    for k, v in _in_maps(inputs)[0].items():
        sim.tensor(k)[:] = v
    sim.simulate()
    return np.array(sim.tensor("out"))


def kernel(**inputs):
    return run(inputs, trace=False)[0]
